# revision 1
# baseline (speedup 1.0000x reference)
"""AugGraphConv (per-relation GAT + lang-level softmax) on 8 TRN2 NeuronCores.

Strategy (dst-sharded graph parallel):
  - Nodes padded to NPAD=50176; core m owns rows [m*6272, (m+1)*6272).
  - Per-core x is host-permuted so owned rows are tiles 0..48 (SPMD program).
  - Stage A (per core, replicated over all nodes): LayerNorm, then per-relation
    feat_r = xn @ [W_r | u_r]  (u_r folds att_src so al = feat[:,128:136]),
    plus ar (att_dst logits) and self path for owned rows.
  - Stage B: edges binned by (own dst tile, relation), chunks of 128 edges.
    Indirect-DMA gather of feat rows by src; one-hot selection matrix S built
    with is_equal vs iota; segment softmax without max-subtraction (logits are
    O(1) bounded): w=exp(leaky(al_src+ar_dst)); num/den accumulate in PSUM via
    S^T matmuls. Padded edges get dst_local=200 -> zero S column -> dropped.
  - Lang stage fused per tile: softmax over 6 feature rows, gelu, residual.
"""

import os
import numpy as np
import ml_dtypes
from contextlib import ExitStack

import concourse.bass as bass
import concourse.mybir as mybir
from concourse.bass import IndirectOffsetOnAxis
from concourse.tile import TileContext
from concourse.bass_utils import run_bass_kernel_spmd

N, D, H, R, C = 50000, 128, 8, 5, 16
P = 128
M = 8
NPAD = 50176            # 392 * 128, divisible by M*P
S = NPAD // M           # 6272 rows per core
T = S // P              # 49 owned tiles per core
GT = NPAD // P          # 392 global tiles
FD = D + H              # 136: [xw | al]
F32 = mybir.dt.float32
BF16 = mybir.dt.bfloat16
I32 = mybir.dt.int32
AF = mybir.ActivationFunctionType
ALU = mybir.AluOpType
AX = mybir.AxisListType
NEGM = -30.0            # softmax mask value (exp(-30) ~ 1e-13, negligible)

LAST_RESULTS = None     # test.py reads exec_time_ns / profile from here


def _split_multiwaits(nc):
    """This toolchain's walrus codegen allows only one sem-wait per
    instruction; hoist extra waits into preceding NoOps on the same engine
    (sequencer executes them in program order, so semantics are identical)."""
    n_split = 0
    for _, bbwrap in nc.bb_map.items():
        bb = bbwrap.bb
        out = []
        changed = False
        for inst in list(bb.instructions):
            si = inst.sync_info
            if si is not None and si.on_wait is not None and len(si.on_wait) > 1:
                waits = list(si.on_wait)
                for w in waits[:-1]:
                    out.append(mybir.InstNoOp(
                        name=nc.get_next_instruction_name(),
                        engine=inst.engine, ins=[], outs=[],
                        sync_info=mybir.SyncInfo(on_wait=[w], on_update=[])))
                    n_split += 1
                si.on_wait = waits[-1:]
                inst.sync_info = si
                changed = True
            out.append(inst)
        if changed:
            bb.instructions = out
    return n_split


def _build(K, TOTC):
    nc = bass.Bass()
    x_full = nc.declare_dram_parameter("x_full", [NPAD, D], F32, isOutput=False)
    srcg = nc.declare_dram_parameter("src_gidx", [P, TOTC], I32, isOutput=False)
    argi = nc.declare_dram_parameter("ar_gidx", [P, TOTC], I32, isOutput=False)
    dstl = nc.declare_dram_parameter("dstl_f", [P, TOTC], BF16, isOutput=False)
    wcat = nc.declare_dram_parameter("wcat", [D, R * FD], BF16, isOutput=False)
    vcat = nc.declare_dram_parameter("vcat", [D, R * H], BF16, isOutput=False)
    wself = nc.declare_dram_parameter("wself", [D, D], BF16, isOutput=False)
    wcross = nc.declare_dram_parameter("wcross", [D, D], F32, isOutput=False)
    asl = nc.declare_dram_parameter("asl_rep", [P, D], F32, isOutput=False)
    adl = nc.declare_dram_parameter("adl_rep", [P, D], F32, isOutput=False)
    bw = nc.declare_dram_parameter("bw_rep", [P, R * D], F32, isOutput=False)
    bl = nc.declare_dram_parameter("bl_rep", [P, D], F32, isOutput=False)
    iota = nc.declare_dram_parameter("iota_f", [P, P], BF16, isOutput=False)
    iden = nc.declare_dram_parameter("ident_f", [P, P], F32, isOutput=False)
    out = nc.declare_dram_parameter("out", [S, D], F32, isOutput=True)

    feat = nc.dram_tensor("feat_all", [R * NPAD, FD], BF16)
    arrel = nc.dram_tensor("ar_rel", [R * S, H], BF16)
    sown = nc.dram_tensor("self_own", [S, D], F32)

    with TileContext(nc) as tc, ExitStack() as ctx:
        cp = ctx.enter_context(tc.tile_pool(name="const", bufs=1))
        sb = ctx.enter_context(tc.tile_pool(name="sb", bufs=3))
        eb = ctx.enter_context(tc.tile_pool(name="eb", bufs=4))
        lb = ctx.enter_context(tc.tile_pool(name="lb", bufs=2))
        psA = ctx.enter_context(tc.tile_pool(name="psA", bufs=2, space="PSUM"))
        psB = ctx.enter_context(tc.tile_pool(name="psB", bufs=1, space="PSUM"))

        # ---- persistent constants / index arrays ----
        wcat_s = cp.tile([D, R * FD], BF16)
        nc.gpsimd.dma_start(out=wcat_s[:], in_=wcat[:])
        vcat_s = cp.tile([D, R * H], BF16)
        nc.gpsimd.dma_start(out=vcat_s[:], in_=vcat[:])
        wself_s = cp.tile([D, D], BF16)
        nc.gpsimd.dma_start(out=wself_s[:], in_=wself[:])
        wcross_s = cp.tile([D, D], F32)
        nc.gpsimd.dma_start(out=wcross_s[:], in_=wcross[:])
        asl_s = cp.tile([P, D], F32)
        nc.gpsimd.dma_start(out=asl_s[:], in_=asl[:])
        adl_s = cp.tile([P, D], F32)
        nc.gpsimd.dma_start(out=adl_s[:], in_=adl[:])
        bw_s = cp.tile([P, R * D], F32)
        nc.gpsimd.dma_start(out=bw_s[:], in_=bw[:])
        bl_s = cp.tile([P, D], F32)
        nc.gpsimd.dma_start(out=bl_s[:], in_=bl[:])
        iota_s = cp.tile([P, P], BF16)
        nc.gpsimd.dma_start(out=iota_s[:], in_=iota[:])
        iden_s = cp.tile([P, P], F32)
        nc.gpsimd.dma_start(out=iden_s[:], in_=iden[:])
        srcg_s = cp.tile([P, TOTC], I32)
        nc.gpsimd.dma_start(out=srcg_s[:], in_=srcg[:])
        argi_s = cp.tile([P, TOTC], I32)
        nc.gpsimd.dma_start(out=argi_s[:], in_=argi[:])
        dstl_s = cp.tile([P, TOTC], BF16)
        nc.gpsimd.dma_start(out=dstl_s[:], in_=dstl[:])

        # ---- Stage A: LN + per-relation features for all nodes ----
        for gt in range(GT):
            xt = sb.tile([P, D], F32, tag="xt")
            nc.gpsimd.dma_start(out=xt[:], in_=x_full[gt * P:(gt + 1) * P, :])
            mu = sb.tile([P, 1], F32, tag="mu")
            nc.vector.tensor_reduce(out=mu[:], in_=xt[:], axis=AX.X, op=ALU.add)
            nc.vector.tensor_scalar_mul(out=mu[:], in0=mu[:], scalar1=1.0 / D)
            xc = sb.tile([P, D], F32, tag="xc")
            nc.vector.tensor_scalar(out=xc[:], in0=xt[:], scalar1=mu[:],
                                    scalar2=None, op0=ALU.subtract)
            sq = sb.tile([P, D], F32, tag="sq")
            nc.scalar.activation(out=sq[:], in_=xc[:], func=AF.Square)
            var = sb.tile([P, 1], F32, tag="var")
            nc.vector.tensor_reduce(out=var[:], in_=sq[:], axis=AX.X, op=ALU.add)
            nc.vector.tensor_scalar(out=var[:], in0=var[:], scalar1=1.0 / D,
                                    scalar2=1e-5, op0=ALU.mult, op1=ALU.add)
            sd = sb.tile([P, 1], F32, tag="sd")
            nc.scalar.activation(out=sd[:], in_=var[:], func=AF.Sqrt)
            rs = sb.tile([P, 1], F32, tag="rs")
            nc.vector.reciprocal(out=rs[:], in_=sd[:])
            xn = sb.tile([P, D], F32, tag="xn")
            nc.vector.tensor_scalar_mul(out=xn[:], in0=xc[:], scalar1=rs[:])
            tp = psA.tile([P, P], F32, tag="tp")
            nc.tensor.transpose(out=tp[:], in_=xn[:], identity=iden_s[:])
            xnT = sb.tile([P, P], BF16, tag="xnT")
            nc.vector.tensor_copy(out=xnT[:], in_=tp[:])
            for r in range(R):
                fm = psA.tile([P, FD], F32, tag="fm")
                nc.tensor.matmul(out=fm[:], lhsT=xnT[:],
                                 rhs=wcat_s[:, r * FD:(r + 1) * FD],
                                 start=True, stop=True)
                fc = sb.tile([P, FD], BF16, tag="fc")
                nc.vector.tensor_copy(out=fc[:], in_=fm[:])
                nc.gpsimd.dma_start(
                    out=feat[r * NPAD + gt * P: r * NPAD + (gt + 1) * P, :],
                    in_=fc[:])
            if gt < T:
                am = psA.tile([P, FD], F32, tag="fm")
                nc.tensor.matmul(out=am[:, :R * H], lhsT=xnT[:], rhs=vcat_s[:],
                                 start=True, stop=True)
                ac = sb.tile([P, R * H], BF16, tag="ac")
                nc.vector.tensor_copy(out=ac[:], in_=am[:, :R * H])
                for r in range(R):
                    nc.gpsimd.dma_start(
                        out=arrel[r * S + gt * P: r * S + (gt + 1) * P, :],
                        in_=ac[:, r * H:(r + 1) * H])
                sm_ = psA.tile([P, FD], F32, tag="fm")
                nc.tensor.matmul(out=sm_[:, :D], lhsT=xnT[:], rhs=wself_s[:],
                                 start=True, stop=True)
                sc = sb.tile([P, D], F32, tag="sc")
                nc.vector.tensor_copy(out=sc[:], in_=sm_[:, :D])
                nc.gpsimd.dma_start(out=sown[gt * P:(gt + 1) * P, :],
                                  in_=sc[:])

        # ---- Stage B: edge aggregation + lang softmax, per owned tile ----
        c = 0
        for t in range(T):
            maskp = lb.tile([P, (R + 1) * H], F32, tag="maskp")
            nc.vector.memset(maskp[:, 0:H], 1.0)
            vts = []
            for r in range(R):
                Kt = K[t][r]
                num_ps = psB.tile([P, D], F32, tag="num")
                den_ps = psB.tile([P, H], F32, tag="den")
                for k in range(Kt):
                    G = eb.tile([P, FD], BF16, tag="G")
                    nc.gpsimd.indirect_dma_start(
                        out=G[:], out_offset=None, in_=feat[:],
                        in_offset=IndirectOffsetOnAxis(ap=srcg_s[:, c:c + 1], axis=0))
                    Aar = eb.tile([P, H], BF16, tag="Aar")
                    nc.gpsimd.indirect_dma_start(
                        out=Aar[:], out_offset=None, in_=arrel[:],
                        in_offset=IndirectOffsetOnAxis(ap=argi_s[:, c:c + 1], axis=0))
                    lg = eb.tile([P, H], F32, tag="lg")
                    nc.vector.tensor_add(out=lg[:], in0=G[:, D:FD], in1=Aar[:])
                    l2 = eb.tile([P, H], F32, tag="l2")
                    nc.vector.tensor_scalar_mul(out=l2[:], in0=lg[:], scalar1=0.2)
                    lr = eb.tile([P, H], F32, tag="lr")
                    nc.vector.tensor_tensor(out=lr[:], in0=lg[:], in1=l2[:],
                                            op=ALU.max)
                    w = eb.tile([P, H], F32, tag="w")
                    nc.scalar.activation(out=w[:], in_=lr[:], func=AF.Exp)
                    wb = eb.tile([P, H], BF16, tag="wb")
                    nc.vector.tensor_copy(out=wb[:], in_=w[:])
                    Sm = eb.tile([P, P], BF16, tag="Sm")
                    nc.vector.tensor_tensor(
                        out=Sm[:], in0=dstl_s[:, c:c + 1].to_broadcast([P, P]),
                        in1=iota_s[:], op=ALU.is_equal)
                    V = eb.tile([P, D], BF16, tag="V")
                    nc.vector.tensor_tensor(
                        out=V[:].rearrange("p (h c) -> p h c", c=C),
                        in0=G[:, 0:D].rearrange("p (h c) -> p h c", c=C),
                        in1=wb[:, :, None].to_broadcast([P, H, C]),
                        op=ALU.mult)
                    nc.tensor.matmul(out=num_ps[:], lhsT=Sm[:], rhs=V[:],
                                     start=(k == 0), stop=(k == Kt - 1))
                    nc.tensor.matmul(out=den_ps[:], lhsT=Sm[:], rhs=wb[:],
                                     start=(k == 0), stop=(k == Kt - 1))
                    c += 1
                den1 = eb.tile([P, H], F32, tag="den1")
                nc.vector.tensor_scalar_max(out=den1[:], in0=den_ps[:],
                                            scalar1=1e-6)
                rec = eb.tile([P, H], F32, tag="rec")
                nc.vector.reciprocal(out=rec[:], in_=den1[:])
                nc.vector.tensor_scalar(
                    out=maskp[:, (r + 1) * H:(r + 2) * H], in0=den_ps[:],
                    scalar1=0.0, scalar2=None, op0=ALU.is_gt)
                O = eb.tile([P, D], F32, tag="O")
                nc.vector.tensor_tensor(
                    out=O[:].rearrange("p (h c) -> p h c", c=C),
                    in0=num_ps[:].rearrange("p (h c) -> p h c", c=C),
                    in1=rec[:, :, None].to_broadcast([P, H, C]),
                    op=ALU.mult)
                nc.vector.tensor_add(out=O[:], in0=O[:],
                                     in1=bw_s[:, r * D:(r + 1) * D])
                g = eb.tile([P, D], F32, tag="g")
                nc.scalar.activation(out=g[:], in_=O[:], func=AF.Gelu)
                tpb = psA.tile([P, P], F32, tag="tp")
                nc.tensor.transpose(out=tpb[:], in_=g[:], identity=iden_s[:])
                gT = eb.tile([P, P], F32, tag="gT")
                nc.vector.tensor_copy(out=gT[:], in_=tpb[:])
                v_ps = psB.tile([P, D], F32, tag="vps")
                nc.tensor.matmul(out=v_ps[:], lhsT=gT[:], rhs=wcross_s[:],
                                 start=True, stop=True)
                vr = lb.tile([P, D], F32, tag=f"v{r + 1}")
                nc.vector.tensor_copy(out=vr[:], in_=v_ps[:])
                vts.append(vr)

            # lang-level GAT over 6 feature rows for this tile
            v0 = lb.tile([P, D], F32, tag="v0")
            nc.gpsimd.dma_start(out=v0[:], in_=sown[t * P:(t + 1) * P, :])
            vall = [v0] + vts
            alp = lb.tile([P, (R + 1) * H], F32, tag="alp")
            tmp = lb.tile([P, D], F32, tag="ltmp")
            for kk in range(R + 1):
                nc.vector.tensor_tensor(out=tmp[:], in0=vall[kk][:],
                                        in1=asl_s[:], op=ALU.mult)
                nc.vector.tensor_reduce(
                    out=alp[:, kk * H:(kk + 1) * H],
                    in_=tmp[:].rearrange("p (h c) -> p h c", c=C),
                    axis=AX.X, op=ALU.add)
            arl = lb.tile([P, H], F32, tag="arl")
            nc.vector.tensor_tensor(out=tmp[:], in0=v0[:], in1=adl_s[:],
                                    op=ALU.mult)
            nc.vector.tensor_reduce(
                out=arl[:], in_=tmp[:].rearrange("p (h c) -> p h c", c=C),
                axis=AX.X, op=ALU.add)
            lgp = lb.tile([P, (R + 1) * H], F32, tag="lgp")
            nc.vector.tensor_tensor(
                out=lgp[:].rearrange("p (k h) -> p k h", h=H),
                in0=alp[:].rearrange("p (k h) -> p k h", h=H),
                in1=arl[:, None, :].to_broadcast([P, R + 1, H]),
                op=ALU.add)
            l2p = lb.tile([P, (R + 1) * H], F32, tag="l2p")
            nc.vector.tensor_scalar_mul(out=l2p[:], in0=lgp[:], scalar1=0.2)
            nc.vector.tensor_tensor(out=lgp[:], in0=lgp[:], in1=l2p[:],
                                    op=ALU.max)
            lm = lb.tile([P, (R + 1) * H], F32, tag="lm")
            nc.vector.tensor_tensor(out=lm[:], in0=lgp[:], in1=maskp[:],
                                    op=ALU.mult)
            mneg = lb.tile([P, (R + 1) * H], F32, tag="mneg")
            nc.vector.tensor_scalar(out=mneg[:], in0=maskp[:], scalar1=1.0,
                                    scalar2=-NEGM, op0=ALU.subtract,
                                    op1=ALU.mult)
            nc.vector.tensor_add(out=lm[:], in0=lm[:], in1=mneg[:])
            ep = lb.tile([P, (R + 1) * H], F32, tag="ep")
            nc.scalar.activation(out=ep[:], in_=lm[:], func=AF.Exp)
            dl = lb.tile([P, H], F32, tag="dl")
            nc.vector.tensor_copy(out=dl[:], in_=ep[:, 0:H])
            for kk in range(1, R + 1):
                nc.vector.tensor_add(out=dl[:], in0=dl[:],
                                     in1=ep[:, kk * H:(kk + 1) * H])
            rl = lb.tile([P, H], F32, tag="rl")
            nc.vector.reciprocal(out=rl[:], in_=dl[:])
            acc = lb.tile([P, D], F32, tag="acc")
            wg = lb.tile([P, H], F32, tag="wg")
            t2 = lb.tile([P, D], F32, tag="t2")
            for kk in range(R + 1):
                nc.vector.tensor_tensor(out=wg[:], in0=ep[:, kk * H:(kk + 1) * H],
                                        in1=rl[:], op=ALU.mult)
                dst_t = acc if kk == 0 else t2
                nc.vector.tensor_tensor(
                    out=dst_t[:].rearrange("p (h c) -> p h c", c=C),
                    in0=vall[kk][:].rearrange("p (h c) -> p h c", c=C),
                    in1=wg[:, :, None].to_broadcast([P, H, C]),
                    op=ALU.mult)
                if kk > 0:
                    nc.vector.tensor_add(out=acc[:], in0=acc[:], in1=t2[:])
            nc.vector.tensor_add(out=acc[:], in0=acc[:], in1=bl_s[:])
            go = lb.tile([P, D], F32, tag="go")
            nc.scalar.activation(out=go[:], in_=acc[:], func=AF.Gelu)
            xr = lb.tile([P, D], F32, tag="xr")
            nc.gpsimd.dma_start(out=xr[:], in_=x_full[t * P:(t + 1) * P, :])
            nc.vector.tensor_add(out=go[:], in0=go[:], in1=xr[:])
            nc.gpsimd.dma_start(out=out[t * P:(t + 1) * P, :], in_=go[:])
    return nc


def _prep(x_inp, edge_index, edge_type, W_self, W_word, att_src_word,
          att_dst_word, bias_word, W_cross, att_src_lang, att_dst_lang,
          bias_lang):
    xpad = np.zeros((NPAD, D), np.float32)
    xpad[:N] = x_inp.astype(np.float32)
    src_all = edge_index[0].astype(np.int64)
    dst_all = edge_index[1].astype(np.int64)
    et_all = edge_type.astype(np.int64)

    # shared params
    Wcat = np.zeros((D, R * FD), np.float32)
    Vcat = np.zeros((D, R * H), np.float32)
    for r in range(R):
        Wr = W_word[r].astype(np.float32)               # [D, D]
        u = np.einsum('dhc,hc->dh', Wr.reshape(D, H, C),
                      att_src_word[r].astype(np.float32))
        v = np.einsum('dhc,hc->dh', Wr.reshape(D, H, C),
                      att_dst_word[r].astype(np.float32))
        Wcat[:, r * FD:r * FD + D] = Wr
        Wcat[:, r * FD + D:(r + 1) * FD] = u
        Vcat[:, r * H:(r + 1) * H] = v
    params = {
        "wcat": Wcat.astype(ml_dtypes.bfloat16),
        "vcat": Vcat.astype(ml_dtypes.bfloat16),
        "wself": W_self.astype(ml_dtypes.bfloat16),
        "wcross": W_cross.astype(np.float32),
        "asl_rep": np.tile(att_src_lang.astype(np.float32).reshape(1, D), (P, 1)),
        "adl_rep": np.tile(att_dst_lang.astype(np.float32).reshape(1, D), (P, 1)),
        "bw_rep": np.tile(bias_word.astype(np.float32).reshape(1, R * D), (P, 1)),
        "bl_rep": np.tile(bias_lang.astype(np.float32).reshape(1, D), (P, 1)),
        "iota_f": np.tile(np.arange(P, dtype=np.float32)[None, :], (P, 1)).astype(ml_dtypes.bfloat16),
        "ident_f": np.eye(P, dtype=np.float32),
    }

    # per-core edge binning
    core_of = dst_all // S
    percore = []
    cnts = np.zeros((M, T, R), np.int64)
    for m in range(M):
        sel = core_of == m
        srcm, dstm, etm = src_all[sel], dst_all[sel], et_all[sel]
        pos = np.empty(NPAD, np.int64)
        pos[m * S:(m + 1) * S] = np.arange(S)
        pos[:m * S] = S + np.arange(m * S)
        pos[(m + 1) * S:] = np.arange((m + 1) * S, NPAD)
        src_l = pos[srcm]
        dst_l = dstm - m * S
        t_loc = dst_l // P
        order = np.lexsort((dst_l % P, etm, t_loc))
        src_l, dst_l, etm, t_loc = (src_l[order], dst_l[order], etm[order],
                                    t_loc[order])
        cnts[m] = np.bincount(t_loc * R + etm, minlength=T * R).reshape(T, R)
        percore.append((pos, src_l, dst_l, etm, t_loc))

    K = np.maximum(1, -(-cnts.max(axis=0) // P))        # [T, R] chunk counts
    TOTC = int(K.sum())
    coff = np.zeros((T, R), np.int64)                    # chunk offsets
    coff.flat[1:] = np.cumsum(K.flat)[:-1]

    in_maps = []
    for m in range(M):
        pos, src_l, dst_l, etm, t_loc = percore[m]
        sg = np.zeros(TOTC * P, np.int32)
        ag = np.zeros(TOTC * P, np.int32)
        dl = np.full(TOTC * P, 200.0, np.float32)
        eoff = np.zeros((T, R), np.int64)
        eoff.flat[1:] = np.cumsum(cnts[m].flat)[:-1]
        for t in range(T):
            for r in range(R):
                n_e = cnts[m, t, r]
                if n_e == 0:
                    continue
                o = eoff[t, r]
                slot = coff[t, r] * P + np.arange(n_e)
                rr = etm[o:o + n_e]
                sg[slot] = rr * NPAD + src_l[o:o + n_e]
                ag[slot] = rr * S + dst_l[o:o + n_e]
                dl[slot] = (dst_l[o:o + n_e] % P).astype(np.float32)
        xperm = np.empty((NPAD, D), np.float32)
        xperm[pos] = xpad
        in_maps.append({
            "x_full": xperm,
            "src_gidx": np.ascontiguousarray(sg.reshape(TOTC, P).T),
            "ar_gidx": np.ascontiguousarray(ag.reshape(TOTC, P).T),
            "dstl_f": np.ascontiguousarray(dl.reshape(TOTC, P).T).astype(ml_dtypes.bfloat16),
            **params,
        })
    return K.tolist(), TOTC, in_maps


def kernel(x_inp, node_type, edge_index, edge_type, W_self, W_word,
           att_src_word, att_dst_word, bias_word, W_cross,
           att_src_lang, att_dst_lang, bias_lang):
    global LAST_RESULTS
    K, TOTC, in_maps = _prep(
        np.asarray(x_inp), np.asarray(edge_index), np.asarray(edge_type),
        np.asarray(W_self), np.asarray(W_word), np.asarray(att_src_word),
        np.asarray(att_dst_word), np.asarray(bias_word), np.asarray(W_cross),
        np.asarray(att_src_lang), np.asarray(att_dst_lang),
        np.asarray(bias_lang))
    nc = _build(K, TOTC)
    _split_multiwaits(nc)
    global LAST_NC, LAST_INMAPS
    LAST_NC, LAST_INMAPS = nc, in_maps
    res = run_bass_kernel_spmd(nc, in_maps, list(range(M)),
                               trace=bool(os.environ.get("BASS_TRACE")))
    LAST_RESULTS = res
    out = np.concatenate([res.results[m]["out"] for m in range(M)], axis=0)
    return out[:N].astype(np.float32)



# revision 3
# speedup vs baseline: 16.7571x; 16.7571x over previous
"""AugGraphConv (per-relation GAT + lang-level softmax) on 8 TRN2 NeuronCores.

v2 — transfer-optimized (the axon tunnel moves ~60-70 MB/s, so host<->device
bytes dominate wall time; device compute is ~ms):

  - x is uploaded as per-core int8 shards [S, D] with per-row absmax scaling
    (LayerNorm is exactly invariant to per-row scale, so no dequant needed),
    then AllGathered on device into the full [NPAD, D] — 8x less H2D than
    replicating x.
  - All weights/attention params are baked into the NEFF as inline consts.
  - Edge indices upload as uint16 src ids + bf16 dst lanes; relation/row
    offsets are reconstructed on device (element_offset gathers + scalar adds).
  - Output is the pre-residual delta in bf16; the f32 residual (+ x_inp) is
    added on host.
  - The shard_map jit is built once per (edges, weights) digest and cached;
    repeat calls skip tracing/lowering. The donated output buffer of call N
    seeds call N+1, so no zero-buffer upload after the first call.

Math per core (dst-sharded, same as v1): LayerNorm, per-relation
feat_r = xn @ [W_r | u_r] for all nodes (u_r folds att_src so al lives in
feat[:, D:FD]); ar logits + self path for owned rows; per-(tile, relation)
edge chunks of 128 use an indirect gather of src feat rows, a one-hot
selection matrix vs iota, and segment softmax without max-subtraction
(logits are O(1)); num/den accumulate in PSUM via S^T matmuls; the lang-level
softmax over the 6 feature rows is fused per owned tile.
"""

import hashlib
import os
import numpy as np
import ml_dtypes
from contextlib import ExitStack

import jax
from jax.sharding import Mesh, PartitionSpec

from jax.experimental.shard_map import shard_map

import concourse.bass as bass
import concourse.mybir as mybir
from concourse.bass import IndirectOffsetOnAxis
from concourse.tile import TileContext
from concourse import bass2jax

N, D, H, R, C = 50000, 128, 8, 5, 16
P = 128
M = 8
NPAD = 50176            # 392 * 128, divisible by M*P
S = NPAD // M           # 6272 rows per core
T = S // P              # 49 owned tiles per core
GT = NPAD // P          # 392 global tiles
FD = D + H              # 136: [xw | al]
ARPAD = 256             # slack rows in arrel so pad-lane gathers stay in-bounds
F32 = mybir.dt.float32
BF16 = mybir.dt.bfloat16
I32 = mybir.dt.int32
I8 = mybir.dt.int8
U16 = mybir.dt.uint16
AF = mybir.ActivationFunctionType
ALU = mybir.AluOpType
AX = mybir.AxisListType
NEGM = -30.0            # lang softmax mask value (exp(-30) ~ 1e-13)

LAST_RESULTS = None


def _split_multiwaits(nc):
    """This toolchain's walrus codegen allows only one sem-wait per
    instruction; hoist extra waits into preceding NoOps on the same engine
    (sequencer executes them in program order, so semantics are identical)."""
    n_split = 0
    for _, bbwrap in nc.bb_map.items():
        bb = bbwrap.bb
        out = []
        changed = False
        for inst in list(bb.instructions):
            si = inst.sync_info
            if si is not None and si.on_wait is not None and len(si.on_wait) > 1:
                waits = list(si.on_wait)
                for w in waits[:-1]:
                    out.append(mybir.InstNoOp(
                        name=nc.get_next_instruction_name(),
                        engine=inst.engine, ins=[], outs=[],
                        sync_info=mybir.SyncInfo(on_wait=[w], on_update=[])))
                    n_split += 1
                si.on_wait = waits[-1:]
                inst.sync_info = si
                changed = True
            out.append(inst)
        if changed:
            bb.instructions = out
    return n_split


def _prep_consts(W_self, W_word, att_src_word, att_dst_word, bias_word,
                 W_cross, att_src_lang, att_dst_lang, bias_lang):
    Wcat = np.zeros((D, R * FD), np.float32)
    Vcat = np.zeros((D, R * H), np.float32)
    for r in range(R):
        Wr = W_word[r].astype(np.float32)               # [D, D]
        u = np.einsum('dhc,hc->dh', Wr.reshape(D, H, C),
                      att_src_word[r].astype(np.float32))
        v = np.einsum('dhc,hc->dh', Wr.reshape(D, H, C),
                      att_dst_word[r].astype(np.float32))
        Wcat[:, r * FD:r * FD + D] = Wr
        Wcat[:, r * FD + D:(r + 1) * FD] = u
        Vcat[:, r * H:(r + 1) * H] = v
    return {
        "wcat": Wcat.astype(ml_dtypes.bfloat16),
        "vcat": Vcat.astype(ml_dtypes.bfloat16),
        "wself": W_self.astype(ml_dtypes.bfloat16),
        "wcross": W_cross.astype(np.float32),
        "asl": np.tile(att_src_lang.astype(np.float32).reshape(1, D), (P, 1)),
        "adl": np.tile(att_dst_lang.astype(np.float32).reshape(1, D), (P, 1)),
        "bw": np.tile(bias_word.astype(np.float32).reshape(1, R * D), (P, 1)),
        "bl": np.tile(bias_lang.astype(np.float32).reshape(1, D), (P, 1)),
        "iota": np.tile(np.arange(P, dtype=np.float32)[None, :],
                        (P, 1)).astype(ml_dtypes.bfloat16),
        "iden": np.eye(P, dtype=np.float32),
    }


def _prep_edges(edge_index, edge_type):
    """Bin edges by (dst core, dst tile, relation); chunk each bin by 128.
    Returns K [T][R] chunk counts, TOTC, and global [M*P, TOTC] index maps."""
    src = edge_index[0].astype(np.int64)
    dst = edge_index[1].astype(np.int64)
    et = edge_type.astype(np.int64)
    E = src.shape[0]
    m = dst // S
    dl = dst - m * S
    t = dl // P
    j = dl - t * P
    binid = (m * T + t) * R + et
    cnt = np.bincount(binid, minlength=M * T * R).reshape(M, T, R)
    K = np.maximum(1, -(-cnt.max(axis=0) // P))          # [T, R]
    TOTC = int(K.sum())
    coff = np.zeros((T, R), np.int64)
    coff.flat[1:] = np.cumsum(K.flat)[:-1]

    order = np.argsort(binid, kind="stable")
    flat_cnt = cnt.reshape(-1)
    starts = np.zeros(M * T * R, np.int64)
    starts[1:] = np.cumsum(flat_cnt)[:-1]
    rank = np.arange(E) - np.repeat(starts, flat_cnt)    # pos within bin
    mo, to, ro = m[order], t[order], et[order]
    slot = coff[to, ro] * P + rank                       # pos within core map

    srcg = np.zeros((M, TOTC * P), np.uint16)
    dstl = np.full((M, TOTC * P), 200.0, np.float32)
    srcg[mo, slot] = src[order].astype(np.uint16)
    dstl[mo, slot] = j[order]
    srcg = np.ascontiguousarray(
        srcg.reshape(M, TOTC, P).transpose(0, 2, 1)).reshape(M * P, TOTC)
    dstl = np.ascontiguousarray(
        dstl.reshape(M, TOTC, P).transpose(0, 2, 1)).astype(
            ml_dtypes.bfloat16).reshape(M * P, TOTC)
    return K.tolist(), TOTC, srcg, dstl


def _build(K, TOTC, consts):
    nc = bass.Bass(num_devices=M)
    x_sh = nc.declare_dram_parameter("x_sh", [S, D], I8, isOutput=False)
    srcg = nc.declare_dram_parameter("srcg", [P, TOTC], U16, isOutput=False)
    dstl = nc.declare_dram_parameter("dstl", [P, TOTC], BF16, isOutput=False)
    dout = nc.declare_dram_parameter("dout", [S, D], BF16, isOutput=True)

    cc_in = nc.dram_tensor("cc_in", [S, D], I8)
    xg = nc.dram_tensor("xg", [NPAD, D], I8, addr_space="Shared")
    feat = nc.dram_tensor("feat_all", [NPAD, R * FD], BF16)
    arrel = nc.dram_tensor("ar_rel", [S + ARPAD, R * H], BF16)

    wcat_c = nc.inline_tensor(consts["wcat"], name="wcat_c")
    vcat_c = nc.inline_tensor(consts["vcat"], name="vcat_c")
    wself_c = nc.inline_tensor(consts["wself"], name="wself_c")
    wcross_c = nc.inline_tensor(consts["wcross"], name="wcross_c")
    asl_c = nc.inline_tensor(consts["asl"], name="asl_c")
    adl_c = nc.inline_tensor(consts["adl"], name="adl_c")
    bw_c = nc.inline_tensor(consts["bw"], name="bw_c")
    bl_c = nc.inline_tensor(consts["bl"], name="bl_c")
    iota_c = nc.inline_tensor(consts["iota"], name="iota_c")
    iden_c = nc.inline_tensor(consts["iden"], name="iden_c")

    with TileContext(nc) as tc, ExitStack() as ctx:
        cp = ctx.enter_context(tc.tile_pool(name="const", bufs=1))
        sb = ctx.enter_context(tc.tile_pool(name="sb", bufs=3))
        eb = ctx.enter_context(tc.tile_pool(name="eb", bufs=4))
        lb = ctx.enter_context(tc.tile_pool(name="lb", bufs=2))
        psA = ctx.enter_context(tc.tile_pool(name="psA", bufs=2, space="PSUM"))
        psB = ctx.enter_context(tc.tile_pool(name="psB", bufs=2, space="PSUM"))

        # ---- persistent constants ----
        wcat_s = cp.tile([D, R * FD], BF16)
        nc.gpsimd.dma_start(out=wcat_s[:], in_=wcat_c[:])
        vcat_s = cp.tile([D, R * H], BF16)
        nc.gpsimd.dma_start(out=vcat_s[:], in_=vcat_c[:])
        wself_s = cp.tile([D, D], BF16)
        nc.gpsimd.dma_start(out=wself_s[:], in_=wself_c[:])
        wcross_s = cp.tile([D, D], F32)
        nc.gpsimd.dma_start(out=wcross_s[:], in_=wcross_c[:])
        asl_s = cp.tile([P, D], F32)
        nc.gpsimd.dma_start(out=asl_s[:], in_=asl_c[:])
        adl_s = cp.tile([P, D], F32)
        nc.gpsimd.dma_start(out=adl_s[:], in_=adl_c[:])
        bw_s = cp.tile([P, R * D], F32)
        nc.gpsimd.dma_start(out=bw_s[:], in_=bw_c[:])
        bl_s = cp.tile([P, D], F32)
        nc.gpsimd.dma_start(out=bl_s[:], in_=bl_c[:])
        iota_s = cp.tile([P, P], BF16)
        nc.gpsimd.dma_start(out=iota_s[:], in_=iota_c[:])
        iden_s = cp.tile([P, P], F32)
        nc.gpsimd.dma_start(out=iden_s[:], in_=iden_c[:])
        srcg_s = cp.tile([P, TOTC], U16)
        nc.gpsimd.dma_start(out=srcg_s[:], in_=srcg[:])
        dstl_s = cp.tile([P, TOTC], BF16)
        nc.gpsimd.dma_start(out=dstl_s[:], in_=dstl[:])
        sown_all = cp.tile([P, T * D], F32)

        # ---- kick off the AllGather of x shards (overlaps local work) ----
        nc.gpsimd.dma_start(out=cc_in[:], in_=x_sh[:])
        nc.gpsimd.collective_compute(
            "AllGather", ALU.bypass,
            replica_groups=[list(range(M))],
            ins=[cc_in[:]], outs=[xg[:]])

        # zero arrel's slack rows (pad-lane gathers read them; keep finite)
        zpad = sb.tile([P, R * H], BF16, tag="zpad")
        nc.vector.memset(zpad[:], 0.0)
        for zi in range(ARPAD // P):
            nc.gpsimd.dma_start(
                out=arrel[S + zi * P:S + (zi + 1) * P, :], in_=zpad[:])

        def layernorm_T(src_dram, row0):
            """int8 rows [P, D] from src_dram -> transposed LN'd bf16 [P, P].
            Per-row int8 scaling cancels in LN (scale-invariant)."""
            xt8 = sb.tile([P, D], I8, tag="xt8")
            nc.gpsimd.dma_start(out=xt8[:], in_=src_dram[row0:row0 + P, :])
            xt = sb.tile([P, D], F32, tag="xt")
            nc.vector.tensor_copy(out=xt[:], in_=xt8[:])
            mu = sb.tile([P, 1], F32, tag="mu")
            nc.vector.tensor_reduce(out=mu[:], in_=xt[:], axis=AX.X, op=ALU.add)
            nc.vector.tensor_scalar_mul(out=mu[:], in0=mu[:], scalar1=1.0 / D)
            xc = sb.tile([P, D], F32, tag="xc")
            nc.vector.tensor_scalar(out=xc[:], in0=xt[:], scalar1=mu[:],
                                    scalar2=None, op0=ALU.subtract)
            sq = sb.tile([P, D], F32, tag="sq")
            nc.scalar.activation(out=sq[:], in_=xc[:], func=AF.Square)
            var = sb.tile([P, 1], F32, tag="var")
            nc.vector.tensor_reduce(out=var[:], in_=sq[:], axis=AX.X,
                                    op=ALU.add)
            nc.vector.tensor_scalar(out=var[:], in0=var[:], scalar1=1.0 / D,
                                    scalar2=1e-5, op0=ALU.mult, op1=ALU.add)
            sd = sb.tile([P, 1], F32, tag="sd")
            nc.scalar.activation(out=sd[:], in_=var[:], func=AF.Sqrt)
            rs = sb.tile([P, 1], F32, tag="rs")
            nc.vector.reciprocal(out=rs[:], in_=sd[:])
            xn = sb.tile([P, D], F32, tag="xn")
            nc.vector.tensor_scalar_mul(out=xn[:], in0=xc[:], scalar1=rs[:])
            tp = psA.tile([P, P], F32, tag="tp")
            nc.tensor.transpose(out=tp[:], in_=xn[:], identity=iden_s[:])
            xnT = sb.tile([P, P], BF16, tag="xnT")
            nc.vector.tensor_copy(out=xnT[:], in_=tp[:])
            return xnT

        # ---- Stage A-own: ar logits + self path for owned rows (local) ----
        for t in range(T):
            xnT = layernorm_T(x_sh, t * P)
            am = psA.tile([P, FD], F32, tag="fm")
            nc.tensor.matmul(out=am[:, :R * H], lhsT=xnT[:], rhs=vcat_s[:],
                             start=True, stop=True)
            ac = sb.tile([P, R * H], BF16, tag="ac")
            nc.vector.tensor_copy(out=ac[:], in_=am[:, :R * H])
            nc.gpsimd.dma_start(out=arrel[t * P:(t + 1) * P, :], in_=ac[:])
            sm_ = psA.tile([P, FD], F32, tag="fm")
            nc.tensor.matmul(out=sm_[:, :D], lhsT=xnT[:], rhs=wself_s[:],
                             start=True, stop=True)
            nc.vector.tensor_copy(out=sown_all[:, t * D:(t + 1) * D],
                                  in_=sm_[:, :D])

        # ---- Stage A-all: per-relation features for all nodes (from xg) ----
        for gt in range(GT):
            xnT = layernorm_T(xg, gt * P)
            for r in range(R):
                fm = psA.tile([P, FD], F32, tag="fm")
                nc.tensor.matmul(out=fm[:], lhsT=xnT[:],
                                 rhs=wcat_s[:, r * FD:(r + 1) * FD],
                                 start=True, stop=True)
                fc = sb.tile([P, FD], BF16, tag="fc")
                nc.vector.tensor_copy(out=fc[:], in_=fm[:])
                nc.gpsimd.dma_start(
                    out=feat[gt * P:(gt + 1) * P, r * FD:(r + 1) * FD],
                    in_=fc[:])

        # ---- Stage B: edge aggregation + lang softmax, per owned tile ----
        c = 0
        for t in range(T):
            maskp = lb.tile([P, (R + 1) * H], F32, tag="maskp")
            nc.vector.memset(maskp[:, 0:H], 1.0)
            vts = []
            for r in range(R):
                Kt = K[t][r]
                nd_ps = psB.tile([P, D + H], F32, tag="nd")
                for k in range(Kt):
                    so32 = eb.tile([P, 1], I32, tag="so32")
                    nc.vector.tensor_copy(out=so32[:], in_=srcg_s[:, c:c + 1])
                    G = eb.tile([P, FD], BF16, tag="G")
                    nc.gpsimd.indirect_dma_start(
                        out=G[:], out_offset=None, in_=feat[:],
                        in_offset=IndirectOffsetOnAxis(ap=so32[:], axis=0),
                        element_offset=r * FD)
                    do32 = eb.tile([P, 1], I32, tag="do32")
                    nc.vector.tensor_scalar(out=do32[:],
                                            in0=dstl_s[:, c:c + 1],
                                            scalar1=float(t * P),
                                            scalar2=None, op0=ALU.add)
                    Aar = eb.tile([P, H], BF16, tag="Aar")
                    nc.gpsimd.indirect_dma_start(
                        out=Aar[:], out_offset=None, in_=arrel[:],
                        in_offset=IndirectOffsetOnAxis(ap=do32[:], axis=0),
                        element_offset=r * H)
                    lg = eb.tile([P, H], F32, tag="lg")
                    nc.vector.tensor_add(out=lg[:], in0=G[:, D:FD], in1=Aar[:])
                    l2 = eb.tile([P, H], F32, tag="l2")
                    nc.vector.tensor_scalar_mul(out=l2[:], in0=lg[:],
                                                scalar1=0.2)
                    lr = eb.tile([P, H], F32, tag="lr")
                    nc.vector.tensor_tensor(out=lr[:], in0=lg[:], in1=l2[:],
                                            op=ALU.max)
                    Vw = eb.tile([P, D + H], BF16, tag="Vw")
                    nc.scalar.activation(out=Vw[:, D:D + H], in_=lr[:],
                                         func=AF.Exp)
                    nc.vector.tensor_tensor(
                        out=Vw[:, 0:D].rearrange("p (h c) -> p h c", c=C),
                        in0=G[:, 0:D].rearrange("p (h c) -> p h c", c=C),
                        in1=Vw[:, D:D + H, None].to_broadcast([P, H, C]),
                        op=ALU.mult)
                    Sm = eb.tile([P, P], BF16, tag="Sm")
                    nc.vector.tensor_tensor(
                        out=Sm[:],
                        in0=dstl_s[:, c:c + 1].to_broadcast([P, P]),
                        in1=iota_s[:], op=ALU.is_equal)
                    nc.tensor.matmul(out=nd_ps[:], lhsT=Sm[:], rhs=Vw[:],
                                     start=(k == 0), stop=(k == Kt - 1))
                    c += 1
                den1 = eb.tile([P, H], F32, tag="den1")
                nc.vector.tensor_scalar_max(out=den1[:], in0=nd_ps[:, D:D + H],
                                            scalar1=1e-6)
                rec = eb.tile([P, H], F32, tag="rec")
                nc.vector.reciprocal(out=rec[:], in_=den1[:])
                nc.vector.tensor_scalar(
                    out=maskp[:, (r + 1) * H:(r + 2) * H],
                    in0=nd_ps[:, D:D + H],
                    scalar1=0.0, scalar2=None, op0=ALU.is_gt)
                O = eb.tile([P, D], F32, tag="O")
                nc.vector.tensor_tensor(
                    out=O[:].rearrange("p (h c) -> p h c", c=C),
                    in0=nd_ps[:, 0:D].rearrange("p (h c) -> p h c", c=C),
                    in1=rec[:, :, None].to_broadcast([P, H, C]),
                    op=ALU.mult)
                nc.vector.tensor_add(out=O[:], in0=O[:],
                                     in1=bw_s[:, r * D:(r + 1) * D])
                g = eb.tile([P, D], F32, tag="g")
                nc.scalar.activation(out=g[:], in_=O[:], func=AF.Gelu)
                tpb = psA.tile([P, P], F32, tag="tp")
                nc.tensor.transpose(out=tpb[:], in_=g[:], identity=iden_s[:])
                gT = eb.tile([P, P], F32, tag="gT")
                nc.vector.tensor_copy(out=gT[:], in_=tpb[:])
                v_ps = psB.tile([P, D], F32, tag="vps")
                nc.tensor.matmul(out=v_ps[:], lhsT=gT[:], rhs=wcross_s[:],
                                 start=True, stop=True)
                vr = lb.tile([P, D], F32, tag=f"v{r + 1}")
                nc.vector.tensor_copy(out=vr[:], in_=v_ps[:])
                vts.append(vr)

            # lang-level GAT over the 6 feature rows for this tile
            v0 = sown_all[:, t * D:(t + 1) * D]
            vall = [v0] + [vr[:] for vr in vts]
            alp = lb.tile([P, (R + 1) * H], F32, tag="alp")
            tmp = lb.tile([P, D], F32, tag="ltmp")
            for kk in range(R + 1):
                nc.vector.tensor_tensor(out=tmp[:], in0=vall[kk],
                                        in1=asl_s[:], op=ALU.mult)
                nc.vector.tensor_reduce(
                    out=alp[:, kk * H:(kk + 1) * H],
                    in_=tmp[:].rearrange("p (h c) -> p h c", c=C),
                    axis=AX.X, op=ALU.add)
            arl = lb.tile([P, H], F32, tag="arl")
            nc.vector.tensor_tensor(out=tmp[:], in0=v0, in1=adl_s[:],
                                    op=ALU.mult)
            nc.vector.tensor_reduce(
                out=arl[:], in_=tmp[:].rearrange("p (h c) -> p h c", c=C),
                axis=AX.X, op=ALU.add)
            lgp = lb.tile([P, (R + 1) * H], F32, tag="lgp")
            nc.vector.tensor_tensor(
                out=lgp[:].rearrange("p (k h) -> p k h", h=H),
                in0=alp[:].rearrange("p (k h) -> p k h", h=H),
                in1=arl[:, None, :].to_broadcast([P, R + 1, H]),
                op=ALU.add)
            l2p = lb.tile([P, (R + 1) * H], F32, tag="l2p")
            nc.vector.tensor_scalar_mul(out=l2p[:], in0=lgp[:], scalar1=0.2)
            nc.vector.tensor_tensor(out=lgp[:], in0=lgp[:], in1=l2p[:],
                                    op=ALU.max)
            lm = lb.tile([P, (R + 1) * H], F32, tag="lm")
            nc.vector.tensor_tensor(out=lm[:], in0=lgp[:], in1=maskp[:],
                                    op=ALU.mult)
            mneg = lb.tile([P, (R + 1) * H], F32, tag="mneg")
            nc.vector.tensor_scalar(out=mneg[:], in0=maskp[:], scalar1=1.0,
                                    scalar2=-NEGM, op0=ALU.subtract,
                                    op1=ALU.mult)
            nc.vector.tensor_add(out=lm[:], in0=lm[:], in1=mneg[:])
            ep = lb.tile([P, (R + 1) * H], F32, tag="ep")
            nc.scalar.activation(out=ep[:], in_=lm[:], func=AF.Exp)
            dl = lb.tile([P, H], F32, tag="dl")
            nc.vector.tensor_copy(out=dl[:], in_=ep[:, 0:H])
            for kk in range(1, R + 1):
                nc.vector.tensor_add(out=dl[:], in0=dl[:],
                                     in1=ep[:, kk * H:(kk + 1) * H])
            rl = lb.tile([P, H], F32, tag="rl")
            nc.vector.reciprocal(out=rl[:], in_=dl[:])
            acc = lb.tile([P, D], F32, tag="acc")
            wg = lb.tile([P, H], F32, tag="wg")
            t2 = lb.tile([P, D], F32, tag="t2")
            for kk in range(R + 1):
                nc.vector.tensor_tensor(out=wg[:],
                                        in0=ep[:, kk * H:(kk + 1) * H],
                                        in1=rl[:], op=ALU.mult)
                dst_t = acc if kk == 0 else t2
                nc.vector.tensor_tensor(
                    out=dst_t[:].rearrange("p (h c) -> p h c", c=C),
                    in0=vall[kk].rearrange("p (h c) -> p h c", c=C),
                    in1=wg[:, :, None].to_broadcast([P, H, C]),
                    op=ALU.mult)
                if kk > 0:
                    nc.vector.tensor_add(out=acc[:], in0=acc[:], in1=t2[:])
            nc.vector.tensor_add(out=acc[:], in0=acc[:], in1=bl_s[:])
            go = lb.tile([P, D], BF16, tag="go")
            nc.scalar.activation(out=go[:], in_=acc[:], func=AF.Gelu)
            nc.gpsimd.dma_start(out=dout[t * P:(t + 1) * P, :], in_=go[:])
    return nc


class _Compiled:
    def __init__(self, sharded, in_names, out_shape, out_dtype, srcg, dstl):
        self.sharded = sharded
        self.in_names = in_names
        self.out_shape = out_shape
        self.out_dtype = out_dtype
        self.srcg = srcg
        self.dstl = dstl
        self.next_zero = None     # device buffer donated as next out seed


_CACHE = {}


def _get_compiled(edge_index, edge_type, weights):
    hasher = hashlib.sha256()
    hasher.update(edge_index.tobytes())
    hasher.update(edge_type.tobytes())
    for w in weights:
        hasher.update(np.ascontiguousarray(w).tobytes())
    key = hasher.hexdigest()
    if key in _CACHE:
        return _CACHE[key]

    consts = _prep_consts(*weights)
    K, TOTC, srcg, dstl = _prep_edges(edge_index, edge_type)
    nc = _build(K, TOTC, consts)
    _split_multiwaits(nc)

    bass2jax.install_neuronx_cc_hook()
    partition_name = (nc.partition_id_tensor.name
                      if nc.partition_id_tensor else None)
    in_names, out_names, out_avals = [], [], []
    for alloc in nc.m.functions[0].allocations:
        if not isinstance(alloc, mybir.MemoryLocationSet):
            continue
        name = alloc.memorylocations[0].name
        if alloc.kind == "ExternalInput":
            if name != partition_name:
                in_names.append(name)
        elif alloc.kind == "ExternalOutput":
            out_names.append(name)
            out_avals.append(jax.core.ShapedArray(
                tuple(alloc.tensor_shape), mybir.dt.np(alloc.dtype)))
    n_params = len(in_names)
    in_names_full = list(in_names) + out_names + (
        [partition_name] if partition_name else [])
    donate = tuple(range(n_params, n_params + len(out_names)))

    def _body(*args):
        operands = list(args)
        if partition_name is not None:
            operands.append(bass2jax.partition_id_tensor())
        outs = bass2jax._bass_exec_p.bind(
            *operands, out_avals=tuple(out_avals),
            in_names=tuple(in_names_full), out_names=tuple(out_names),
            lowering_input_output_aliases=(), sim_require_finite=True,
            sim_require_nnan=True, nc=nc)
        return tuple(outs)

    devices = jax.devices()[:M]
    mesh = Mesh(np.asarray(devices), ("core",))
    in_specs = (PartitionSpec("core"),) * (n_params + len(out_names))
    out_specs = (PartitionSpec("core"),) * len(out_names)
    sharded = jax.jit(
        shard_map(_body, mesh=mesh, in_specs=in_specs, out_specs=out_specs,
                  check_rep=False),
        donate_argnums=donate, keep_unused=True)

    comp = _Compiled(sharded, in_names,
                     tuple(out_avals[0].shape), out_avals[0].dtype,
                     srcg, dstl)
    _CACHE[key] = comp
    return comp


def kernel(x_inp, node_type, edge_index, edge_type, W_self, W_word,
           att_src_word, att_dst_word, bias_word, W_cross,
           att_src_lang, att_dst_lang, bias_lang):
    global LAST_RESULTS
    x_inp = np.asarray(x_inp, dtype=np.float32)
    comp = _get_compiled(
        np.asarray(edge_index), np.asarray(edge_type),
        [np.asarray(W_self), np.asarray(W_word), np.asarray(att_src_word),
         np.asarray(att_dst_word), np.asarray(bias_word), np.asarray(W_cross),
         np.asarray(att_src_lang), np.asarray(att_dst_lang),
         np.asarray(bias_lang)])

    # per-row int8 quantization of x (LayerNorm is scale-invariant per row)
    xpad = np.zeros((NPAD, D), np.float32)
    xpad[:N] = x_inp
    amax = np.abs(xpad).max(axis=1, keepdims=True)
    scale = np.where(amax > 0, 127.0 / np.maximum(amax, 1e-30), 0.0)
    q = np.clip(np.rint(xpad * scale), -127, 127).astype(np.int8)

    by_name = {"x_sh": q, "srcg": comp.srcg, "dstl": comp.dstl}
    args = [by_name[n] for n in comp.in_names]
    if comp.next_zero is None:
        seed = np.zeros((M * comp.out_shape[0],) + comp.out_shape[1:],
                        comp.out_dtype)
    else:
        seed = comp.next_zero
    out_arrs = comp.sharded(*args, seed)
    comp.next_zero = out_arrs[0]
    delta = np.asarray(out_arrs[0]).astype(np.float32)   # [M*S, D]
    LAST_RESULTS = None
    out = delta[:N] + x_inp
    return out


# revision 8
# speedup vs baseline: 24.3979x; 1.4560x over previous
"""AugGraphConv (per-relation GAT + lang-level softmax) on 8 TRN2 NeuronCores.

v2 — transfer-optimized (the axon tunnel moves ~60-70 MB/s, so host<->device
bytes dominate wall time; device compute is ~ms):

  - x is uploaded as per-core int8 shards [S, D] with per-row absmax scaling
    (LayerNorm is exactly invariant to per-row scale, so no dequant needed),
    then AllGathered on device into the full [NPAD, D] — 8x less H2D than
    replicating x.
  - All weights/attention params are baked into the NEFF as inline consts.
  - Edge indices upload as uint16 src ids + bf16 dst lanes; relation/row
    offsets are reconstructed on device (element_offset gathers + scalar adds).
  - Output is the pre-residual delta in bf16; the f32 residual (+ x_inp) is
    added on host.
  - The shard_map jit is built once per (edges, weights) digest and cached;
    repeat calls skip tracing/lowering. The donated output buffer of call N
    seeds call N+1, so no zero-buffer upload after the first call.

Math per core (dst-sharded, same as v1): LayerNorm, per-relation
feat_r = xn @ [W_r | u_r] for all nodes (u_r folds att_src so al lives in
feat[:, D:FD]); ar logits + self path for owned rows; per-(tile, relation)
edge chunks of 128 use an indirect gather of src feat rows, a one-hot
selection matrix vs iota, and segment softmax without max-subtraction
(logits are O(1)); num/den accumulate in PSUM via S^T matmuls; the lang-level
softmax over the 6 feature rows is fused per owned tile.
"""

import hashlib
import os
import numpy as np
import ml_dtypes
from contextlib import ExitStack

import jax
from jax.sharding import Mesh, PartitionSpec

from jax.experimental.shard_map import shard_map

import concourse.bass as bass
import concourse.mybir as mybir
from concourse.bass import IndirectOffsetOnAxis
from concourse.tile import TileContext
from concourse import bass2jax

N, D, H, R, C = 50000, 128, 8, 5, 16
P = 128
M = 8
NPAD = 50176            # 392 * 128, divisible by M*P
S = NPAD // M           # 6272 rows per core
T = S // P              # 49 owned tiles per core
GT = NPAD // P          # 392 global tiles
FD = D + H              # 136: [xw | al]
ARPAD = 256             # slack rows in arrel so pad-lane gathers stay in-bounds
F32 = mybir.dt.float32
BF16 = mybir.dt.bfloat16
I32 = mybir.dt.int32
I8 = mybir.dt.int8
U16 = mybir.dt.uint16
AF = mybir.ActivationFunctionType
ALU = mybir.AluOpType
AX = mybir.AxisListType
NEGM = -30.0            # lang softmax mask value (exp(-30) ~ 1e-13)

LAST_RESULTS = None


def _split_multiwaits(nc):
    """This toolchain's walrus codegen allows only one sem-wait per
    instruction; hoist extra waits into preceding NoOps on the same engine
    (sequencer executes them in program order, so semantics are identical)."""
    n_split = 0
    for _, bbwrap in nc.bb_map.items():
        bb = bbwrap.bb
        out = []
        changed = False
        for inst in list(bb.instructions):
            si = inst.sync_info
            if si is not None and si.on_wait is not None and len(si.on_wait) > 1:
                waits = list(si.on_wait)
                for w in waits[:-1]:
                    out.append(mybir.InstNoOp(
                        name=nc.get_next_instruction_name(),
                        engine=inst.engine, ins=[], outs=[],
                        sync_info=mybir.SyncInfo(on_wait=[w], on_update=[])))
                    n_split += 1
                si.on_wait = waits[-1:]
                inst.sync_info = si
                changed = True
            out.append(inst)
        if changed:
            bb.instructions = out
    return n_split


def _prep_consts(W_self, W_word, att_src_word, att_dst_word, bias_word,
                 W_cross, att_src_lang, att_dst_lang, bias_lang):
    Wcat = np.zeros((D, R * FD), np.float32)
    Vcat = np.zeros((D, R * H), np.float32)
    for r in range(R):
        Wr = W_word[r].astype(np.float32)               # [D, D]
        u = np.einsum('dhc,hc->dh', Wr.reshape(D, H, C),
                      att_src_word[r].astype(np.float32))
        v = np.einsum('dhc,hc->dh', Wr.reshape(D, H, C),
                      att_dst_word[r].astype(np.float32))
        Wcat[:, r * FD:r * FD + D] = Wr
        Wcat[:, r * FD + D:(r + 1) * FD] = u
        Vcat[:, r * H:(r + 1) * H] = v
    return {
        "wcat": Wcat.astype(ml_dtypes.bfloat16),
        "vcat": Vcat.astype(ml_dtypes.bfloat16),
        "wself": W_self.astype(ml_dtypes.bfloat16),
        "wcross": W_cross.astype(np.float32),
        "asl": np.tile(att_src_lang.astype(np.float32).reshape(1, D), (P, 1)),
        "adl": np.tile(att_dst_lang.astype(np.float32).reshape(1, D), (P, 1)),
        "bw": np.tile(bias_word.astype(np.float32).reshape(1, R * D), (P, 1)),
        "bl": np.tile(bias_lang.astype(np.float32).reshape(1, D), (P, 1)),
        "iota": np.tile(np.arange(P, dtype=np.float32)[None, :],
                        (P, 1)).astype(ml_dtypes.bfloat16),
        "iden": np.eye(P, dtype=np.float32),
    }


def _prep_edges(edge_index, edge_type):
    """Bin edges by (dst core, dst tile, relation); chunk each bin by 128.
    Returns K [T][R] chunk counts, TOTC, and global [M*P, TOTC] index maps."""
    src = edge_index[0].astype(np.int64)
    dst = edge_index[1].astype(np.int64)
    et = edge_type.astype(np.int64)
    E = src.shape[0]
    m = dst // S
    dl = dst - m * S
    t = dl // P
    j = dl - t * P
    binid = (m * T + t) * R + et
    cnt = np.bincount(binid, minlength=M * T * R).reshape(M, T, R)
    K = np.maximum(1, -(-cnt.max(axis=0) // P))          # [T, R]
    TOTC = int(K.sum())
    coff = np.zeros((T, R), np.int64)
    coff.flat[1:] = np.cumsum(K.flat)[:-1]

    order = np.argsort(binid, kind="stable")
    flat_cnt = cnt.reshape(-1)
    starts = np.zeros(M * T * R, np.int64)
    starts[1:] = np.cumsum(flat_cnt)[:-1]
    rank = np.arange(E) - np.repeat(starts, flat_cnt)    # pos within bin
    mo, to, ro = m[order], t[order], et[order]
    slot = coff[to, ro] * P + rank                       # pos within core map

    srcg = np.zeros((M, TOTC * P), np.uint16)
    dstl = np.full((M, TOTC * P), 200.0, np.float32)
    srcg[mo, slot] = src[order].astype(np.uint16)
    dstl[mo, slot] = j[order]
    srcg = np.ascontiguousarray(
        srcg.reshape(M, TOTC, P).transpose(0, 2, 1)).reshape(M * P, TOTC)
    dstl = np.ascontiguousarray(
        dstl.reshape(M, TOTC, P).transpose(0, 2, 1)).astype(
            ml_dtypes.bfloat16).reshape(M * P, TOTC)
    return K.tolist(), TOTC, srcg, dstl


def _build(K, TOTC, consts):
    nc = bass.Bass(num_devices=M)
    x_sh = nc.declare_dram_parameter("x_sh", [S, D], I8, isOutput=False)
    srcg = nc.declare_dram_parameter("srcg", [P, TOTC], U16, isOutput=False)
    dstl = nc.declare_dram_parameter("dstl", [P, TOTC], BF16, isOutput=False)
    dout = nc.declare_dram_parameter("dout", [S, D], I8, isOutput=True)
    dsc = nc.declare_dram_parameter("dsc", [S, 1], F32, isOutput=True)

    cc_in = nc.dram_tensor("cc_in", [S, D], I8)
    xg = nc.dram_tensor("xg", [NPAD, D], I8, addr_space="Shared")
    feat = nc.dram_tensor("feat_all", [NPAD, R * FD], BF16)
    arrel = nc.dram_tensor("ar_rel", [S + ARPAD, R * H], BF16)

    wcat_c = nc.inline_tensor(consts["wcat"], name="wcat_c")
    vcat_c = nc.inline_tensor(consts["vcat"], name="vcat_c")
    wself_c = nc.inline_tensor(consts["wself"], name="wself_c")
    wcross_c = nc.inline_tensor(consts["wcross"], name="wcross_c")
    asl_c = nc.inline_tensor(consts["asl"], name="asl_c")
    adl_c = nc.inline_tensor(consts["adl"], name="adl_c")
    bw_c = nc.inline_tensor(consts["bw"], name="bw_c")
    bl_c = nc.inline_tensor(consts["bl"], name="bl_c")
    iota_c = nc.inline_tensor(consts["iota"], name="iota_c")
    iden_c = nc.inline_tensor(consts["iden"], name="iden_c")

    with TileContext(nc) as tc, ExitStack() as ctx:
        cp = ctx.enter_context(tc.tile_pool(name="const", bufs=1))
        sb = ctx.enter_context(tc.tile_pool(name="sb", bufs=3))
        eb = ctx.enter_context(tc.tile_pool(name="eb", bufs=4))
        lb = ctx.enter_context(tc.tile_pool(name="lb", bufs=2))
        psA = ctx.enter_context(tc.tile_pool(name="psA", bufs=2, space="PSUM"))
        psB = ctx.enter_context(tc.tile_pool(name="psB", bufs=2, space="PSUM"))

        # ---- persistent constants ----
        wcat_s = cp.tile([D, R * FD], BF16)
        nc.gpsimd.dma_start(out=wcat_s[:], in_=wcat_c[:])
        vcat_s = cp.tile([D, R * H], BF16)
        nc.gpsimd.dma_start(out=vcat_s[:], in_=vcat_c[:])
        wself_s = cp.tile([D, D], BF16)
        nc.gpsimd.dma_start(out=wself_s[:], in_=wself_c[:])
        wcross_s = cp.tile([D, D], F32)
        nc.gpsimd.dma_start(out=wcross_s[:], in_=wcross_c[:])
        asl_s = cp.tile([P, D], F32)
        nc.gpsimd.dma_start(out=asl_s[:], in_=asl_c[:])
        adl_s = cp.tile([P, D], F32)
        nc.gpsimd.dma_start(out=adl_s[:], in_=adl_c[:])
        bw_s = cp.tile([P, R * D], F32)
        nc.gpsimd.dma_start(out=bw_s[:], in_=bw_c[:])
        bl_s = cp.tile([P, D], F32)
        nc.gpsimd.dma_start(out=bl_s[:], in_=bl_c[:])
        iota_s = cp.tile([P, P], BF16)
        nc.gpsimd.dma_start(out=iota_s[:], in_=iota_c[:])
        iden_s = cp.tile([P, P], F32)
        nc.gpsimd.dma_start(out=iden_s[:], in_=iden_c[:])
        srcg_s = cp.tile([P, TOTC], U16)
        nc.gpsimd.dma_start(out=srcg_s[:], in_=srcg[:])
        dstl_s = cp.tile([P, TOTC], BF16)
        nc.gpsimd.dma_start(out=dstl_s[:], in_=dstl[:])
        sown_all = cp.tile([P, T * D], F32)

        # ---- kick off the AllGather of x shards (overlaps local work) ----
        nc.gpsimd.dma_start(out=cc_in[:], in_=x_sh[:])
        nc.gpsimd.collective_compute(
            "AllGather", ALU.bypass,
            replica_groups=[list(range(M))],
            ins=[cc_in[:]], outs=[xg[:]])

        # zero arrel's slack rows (pad-lane gathers read them; keep finite)
        zpad = sb.tile([P, R * H], BF16, tag="zpad")
        nc.vector.memset(zpad[:], 0.0)
        for zi in range(ARPAD // P):
            nc.gpsimd.dma_start(
                out=arrel[S + zi * P:S + (zi + 1) * P, :], in_=zpad[:])

        def layernorm_T(src_dram, row0):
            """int8 rows [P, D] from src_dram -> transposed LN'd bf16 [P, P].
            Per-row int8 scaling cancels in LN (scale-invariant)."""
            xt8 = sb.tile([P, D], I8, tag="xt8")
            nc.gpsimd.dma_start(out=xt8[:], in_=src_dram[row0:row0 + P, :])
            xt = sb.tile([P, D], F32, tag="xt")
            nc.vector.tensor_copy(out=xt[:], in_=xt8[:])
            mu = sb.tile([P, 1], F32, tag="mu")
            nc.vector.tensor_reduce(out=mu[:], in_=xt[:], axis=AX.X, op=ALU.add)
            nc.vector.tensor_scalar_mul(out=mu[:], in0=mu[:], scalar1=1.0 / D)
            xc = sb.tile([P, D], F32, tag="xc")
            nc.vector.tensor_scalar(out=xc[:], in0=xt[:], scalar1=mu[:],
                                    scalar2=None, op0=ALU.subtract)
            sq = sb.tile([P, D], F32, tag="sq")
            nc.scalar.activation(out=sq[:], in_=xc[:], func=AF.Square)
            var = sb.tile([P, 1], F32, tag="var")
            nc.vector.tensor_reduce(out=var[:], in_=sq[:], axis=AX.X,
                                    op=ALU.add)
            nc.vector.tensor_scalar(out=var[:], in0=var[:], scalar1=1.0 / D,
                                    scalar2=1e-5, op0=ALU.mult, op1=ALU.add)
            sd = sb.tile([P, 1], F32, tag="sd")
            nc.scalar.activation(out=sd[:], in_=var[:], func=AF.Sqrt)
            rs = sb.tile([P, 1], F32, tag="rs")
            nc.vector.reciprocal(out=rs[:], in_=sd[:])
            xn = sb.tile([P, D], F32, tag="xn")
            nc.vector.tensor_scalar_mul(out=xn[:], in0=xc[:], scalar1=rs[:])
            tp = psA.tile([P, P], F32, tag="tp")
            nc.tensor.transpose(out=tp[:], in_=xn[:], identity=iden_s[:])
            xnT = sb.tile([P, P], BF16, tag="xnT")
            nc.vector.tensor_copy(out=xnT[:], in_=tp[:])
            return xnT

        # ---- Stage A-own: ar logits + self path for owned rows (local) ----
        for t in range(T):
            xnT = layernorm_T(x_sh, t * P)
            am = psA.tile([P, FD], F32, tag="fm")
            nc.tensor.matmul(out=am[:, :R * H], lhsT=xnT[:], rhs=vcat_s[:],
                             start=True, stop=True)
            ac = sb.tile([P, R * H], BF16, tag="ac")
            nc.vector.tensor_copy(out=ac[:], in_=am[:, :R * H])
            nc.gpsimd.dma_start(out=arrel[t * P:(t + 1) * P, :], in_=ac[:])
            sm_ = psA.tile([P, FD], F32, tag="fm")
            nc.tensor.matmul(out=sm_[:, :D], lhsT=xnT[:], rhs=wself_s[:],
                             start=True, stop=True)
            nc.vector.tensor_copy(out=sown_all[:, t * D:(t + 1) * D],
                                  in_=sm_[:, :D])

        # ---- Stage A-all: per-relation features for all nodes (from xg) ----
        for gt in range(GT):
            xnT = layernorm_T(xg, gt * P)
            for r in range(R):
                fm = psA.tile([P, FD], F32, tag="fm")
                nc.tensor.matmul(out=fm[:], lhsT=xnT[:],
                                 rhs=wcat_s[:, r * FD:(r + 1) * FD],
                                 start=True, stop=True)
                fc = sb.tile([P, FD], BF16, tag="fc")
                nc.vector.tensor_copy(out=fc[:], in_=fm[:])
                nc.gpsimd.dma_start(
                    out=feat[gt * P:(gt + 1) * P, r * FD:(r + 1) * FD],
                    in_=fc[:])

        # ---- Stage B: edge aggregation + lang softmax, per owned tile ----
        c = 0
        for t in range(T):
            maskp = lb.tile([P, (R + 1) * H], F32, tag="maskp")
            nc.vector.memset(maskp[:, 0:H], 1.0)
            vts = []
            for r in range(R):
                Kt = K[t][r]
                nd_ps = psB.tile([P, D + H], F32, tag="nd")
                for k in range(Kt):
                    so32 = eb.tile([P, 1], I32, tag="so32")
                    nc.vector.tensor_copy(out=so32[:], in_=srcg_s[:, c:c + 1])
                    G = eb.tile([P, FD], BF16, tag="G")
                    nc.gpsimd.indirect_dma_start(
                        out=G[:], out_offset=None, in_=feat[:],
                        in_offset=IndirectOffsetOnAxis(ap=so32[:], axis=0),
                        element_offset=r * FD)
                    do32 = eb.tile([P, 1], I32, tag="do32")
                    nc.vector.tensor_scalar(out=do32[:],
                                            in0=dstl_s[:, c:c + 1],
                                            scalar1=float(t * P),
                                            scalar2=None, op0=ALU.add)
                    Aar = eb.tile([P, H], BF16, tag="Aar")
                    nc.gpsimd.indirect_dma_start(
                        out=Aar[:], out_offset=None, in_=arrel[:],
                        in_offset=IndirectOffsetOnAxis(ap=do32[:], axis=0),
                        element_offset=r * H)
                    lg = eb.tile([P, H], F32, tag="lg")
                    nc.vector.tensor_add(out=lg[:], in0=G[:, D:FD], in1=Aar[:])
                    l2 = eb.tile([P, H], F32, tag="l2")
                    nc.vector.tensor_scalar_mul(out=l2[:], in0=lg[:],
                                                scalar1=0.2)
                    lr = eb.tile([P, H], F32, tag="lr")
                    nc.vector.tensor_tensor(out=lr[:], in0=lg[:], in1=l2[:],
                                            op=ALU.max)
                    Vw = eb.tile([P, D + H], BF16, tag="Vw")
                    nc.scalar.activation(out=Vw[:, D:D + H], in_=lr[:],
                                         func=AF.Exp)
                    nc.vector.tensor_tensor(
                        out=Vw[:, 0:D].rearrange("p (h c) -> p h c", c=C),
                        in0=G[:, 0:D].rearrange("p (h c) -> p h c", c=C),
                        in1=Vw[:, D:D + H, None].to_broadcast([P, H, C]),
                        op=ALU.mult)
                    Sm = eb.tile([P, P], BF16, tag="Sm")
                    nc.vector.tensor_tensor(
                        out=Sm[:],
                        in0=dstl_s[:, c:c + 1].to_broadcast([P, P]),
                        in1=iota_s[:], op=ALU.is_equal)
                    nc.tensor.matmul(out=nd_ps[:], lhsT=Sm[:], rhs=Vw[:],
                                     start=(k == 0), stop=(k == Kt - 1))
                    c += 1
                den1 = eb.tile([P, H], F32, tag="den1")
                nc.vector.tensor_scalar_max(out=den1[:], in0=nd_ps[:, D:D + H],
                                            scalar1=1e-6)
                rec = eb.tile([P, H], F32, tag="rec")
                nc.vector.reciprocal(out=rec[:], in_=den1[:])
                nc.vector.tensor_scalar(
                    out=maskp[:, (r + 1) * H:(r + 2) * H],
                    in0=nd_ps[:, D:D + H],
                    scalar1=0.0, scalar2=None, op0=ALU.is_gt)
                O = eb.tile([P, D], F32, tag="O")
                nc.vector.tensor_tensor(
                    out=O[:].rearrange("p (h c) -> p h c", c=C),
                    in0=nd_ps[:, 0:D].rearrange("p (h c) -> p h c", c=C),
                    in1=rec[:, :, None].to_broadcast([P, H, C]),
                    op=ALU.mult)
                nc.vector.tensor_add(out=O[:], in0=O[:],
                                     in1=bw_s[:, r * D:(r + 1) * D])
                g = eb.tile([P, D], F32, tag="g")
                nc.scalar.activation(out=g[:], in_=O[:], func=AF.Gelu)
                tpb = psA.tile([P, P], F32, tag="tp")
                nc.tensor.transpose(out=tpb[:], in_=g[:], identity=iden_s[:])
                gT = eb.tile([P, P], F32, tag="gT")
                nc.vector.tensor_copy(out=gT[:], in_=tpb[:])
                v_ps = psB.tile([P, D], F32, tag="vps")
                nc.tensor.matmul(out=v_ps[:], lhsT=gT[:], rhs=wcross_s[:],
                                 start=True, stop=True)
                vr = lb.tile([P, D], F32, tag=f"v{r + 1}")
                nc.vector.tensor_copy(out=vr[:], in_=v_ps[:])
                vts.append(vr)

            # lang-level GAT over the 6 feature rows for this tile
            v0 = sown_all[:, t * D:(t + 1) * D]
            vall = [v0] + [vr[:] for vr in vts]
            alp = lb.tile([P, (R + 1) * H], F32, tag="alp")
            tmp = lb.tile([P, D], F32, tag="ltmp")
            for kk in range(R + 1):
                nc.vector.tensor_tensor(out=tmp[:], in0=vall[kk],
                                        in1=asl_s[:], op=ALU.mult)
                nc.vector.tensor_reduce(
                    out=alp[:, kk * H:(kk + 1) * H],
                    in_=tmp[:].rearrange("p (h c) -> p h c", c=C),
                    axis=AX.X, op=ALU.add)
            arl = lb.tile([P, H], F32, tag="arl")
            nc.vector.tensor_tensor(out=tmp[:], in0=v0, in1=adl_s[:],
                                    op=ALU.mult)
            nc.vector.tensor_reduce(
                out=arl[:], in_=tmp[:].rearrange("p (h c) -> p h c", c=C),
                axis=AX.X, op=ALU.add)
            lgp = lb.tile([P, (R + 1) * H], F32, tag="lgp")
            nc.vector.tensor_tensor(
                out=lgp[:].rearrange("p (k h) -> p k h", h=H),
                in0=alp[:].rearrange("p (k h) -> p k h", h=H),
                in1=arl[:, None, :].to_broadcast([P, R + 1, H]),
                op=ALU.add)
            l2p = lb.tile([P, (R + 1) * H], F32, tag="l2p")
            nc.vector.tensor_scalar_mul(out=l2p[:], in0=lgp[:], scalar1=0.2)
            nc.vector.tensor_tensor(out=lgp[:], in0=lgp[:], in1=l2p[:],
                                    op=ALU.max)
            lm = lb.tile([P, (R + 1) * H], F32, tag="lm")
            nc.vector.tensor_tensor(out=lm[:], in0=lgp[:], in1=maskp[:],
                                    op=ALU.mult)
            mneg = lb.tile([P, (R + 1) * H], F32, tag="mneg")
            nc.vector.tensor_scalar(out=mneg[:], in0=maskp[:], scalar1=1.0,
                                    scalar2=-NEGM, op0=ALU.subtract,
                                    op1=ALU.mult)
            nc.vector.tensor_add(out=lm[:], in0=lm[:], in1=mneg[:])
            ep = lb.tile([P, (R + 1) * H], F32, tag="ep")
            nc.scalar.activation(out=ep[:], in_=lm[:], func=AF.Exp)
            dl = lb.tile([P, H], F32, tag="dl")
            nc.vector.tensor_copy(out=dl[:], in_=ep[:, 0:H])
            for kk in range(1, R + 1):
                nc.vector.tensor_add(out=dl[:], in0=dl[:],
                                     in1=ep[:, kk * H:(kk + 1) * H])
            rl = lb.tile([P, H], F32, tag="rl")
            nc.vector.reciprocal(out=rl[:], in_=dl[:])
            acc = lb.tile([P, D], F32, tag="acc")
            wg = lb.tile([P, H], F32, tag="wg")
            t2 = lb.tile([P, D], F32, tag="t2")
            for kk in range(R + 1):
                nc.vector.tensor_tensor(out=wg[:],
                                        in0=ep[:, kk * H:(kk + 1) * H],
                                        in1=rl[:], op=ALU.mult)
                dst_t = acc if kk == 0 else t2
                nc.vector.tensor_tensor(
                    out=dst_t[:].rearrange("p (h c) -> p h c", c=C),
                    in0=vall[kk].rearrange("p (h c) -> p h c", c=C),
                    in1=wg[:, :, None].to_broadcast([P, H, C]),
                    op=ALU.mult)
                if kk > 0:
                    nc.vector.tensor_add(out=acc[:], in0=acc[:], in1=t2[:])
            nc.vector.tensor_add(out=acc[:], in0=acc[:], in1=bl_s[:])
            go = lb.tile([P, D], F32, tag="go")
            nc.scalar.activation(out=go[:], in_=acc[:], func=AF.Gelu)
            # per-row int8 quantization of the delta; scale rides along
            ab = lb.tile([P, D], F32, tag="ab")
            nc.scalar.activation(out=ab[:], in_=go[:], func=AF.Abs)
            rmax = lb.tile([P, 1], F32, tag="rmax")
            nc.vector.tensor_reduce(out=rmax[:], in_=ab[:], axis=AX.X,
                                    op=ALU.max)
            nc.vector.tensor_scalar_max(out=rmax[:], in0=rmax[:],
                                        scalar1=1e-20)
            rsc = lb.tile([P, 1], F32, tag="rsc")
            nc.vector.reciprocal(out=rsc[:], in_=rmax[:])
            nc.vector.tensor_scalar_mul(out=rsc[:], in0=rsc[:],
                                        scalar1=126.99)
            qo = lb.tile([P, D], I8, tag="qo")
            nc.vector.tensor_scalar_mul(out=qo[:], in0=go[:], scalar1=rsc[:])
            nc.gpsimd.dma_start(out=dout[t * P:(t + 1) * P, :], in_=qo[:])
            nc.gpsimd.dma_start(out=dsc[t * P:(t + 1) * P, :], in_=rmax[:])
    return nc


class _Compiled:
    def __init__(self, sharded, in_names, out_avals, srcg_dev, dstl_dev):
        self.sharded = sharded
        self.in_names = in_names
        self.out_avals = out_avals
        self.srcg_dev = srcg_dev   # device-resident, never donated
        self.dstl_dev = dstl_dev
        self.next_seed = None      # device buffers donated as next out seeds
        self.q_buf = np.zeros((NPAD, D), np.int8)
        self.xtmp = np.empty((N, D), np.float32)


_CACHE = {}


def _get_compiled(edge_index, edge_type, weights):
    hasher = hashlib.sha256()
    hasher.update(edge_index.tobytes())
    hasher.update(edge_type.tobytes())
    for w in weights:
        hasher.update(np.ascontiguousarray(w).tobytes())
    key = hasher.hexdigest()
    if key in _CACHE:
        return _CACHE[key]

    consts = _prep_consts(*weights)
    K, TOTC, srcg, dstl = _prep_edges(edge_index, edge_type)
    nc = _build(K, TOTC, consts)
    _split_multiwaits(nc)

    bass2jax.install_neuronx_cc_hook()
    partition_name = (nc.partition_id_tensor.name
                      if nc.partition_id_tensor else None)
    in_names, out_names, out_avals = [], [], []
    for alloc in nc.m.functions[0].allocations:
        if not isinstance(alloc, mybir.MemoryLocationSet):
            continue
        name = alloc.memorylocations[0].name
        if alloc.kind == "ExternalInput":
            if name != partition_name:
                in_names.append(name)
        elif alloc.kind == "ExternalOutput":
            out_names.append(name)
            out_avals.append(jax.core.ShapedArray(
                tuple(alloc.tensor_shape), mybir.dt.np(alloc.dtype)))
    n_params = len(in_names)
    in_names_full = list(in_names) + out_names + (
        [partition_name] if partition_name else [])
    donate = tuple(range(n_params, n_params + len(out_names)))

    def _body(*args):
        operands = list(args)
        if partition_name is not None:
            operands.append(bass2jax.partition_id_tensor())
        outs = bass2jax._bass_exec_p.bind(
            *operands, out_avals=tuple(out_avals),
            in_names=tuple(in_names_full), out_names=tuple(out_names),
            lowering_input_output_aliases=(), sim_require_finite=True,
            sim_require_nnan=True, nc=nc)
        return tuple(outs)

    devices = jax.devices()[:M]
    mesh = Mesh(np.asarray(devices), ("core",))
    in_specs = (PartitionSpec("core"),) * (n_params + len(out_names))
    out_specs = (PartitionSpec("core"),) * len(out_names)
    sharded = jax.jit(
        shard_map(_body, mesh=mesh, in_specs=in_specs, out_specs=out_specs,
                  check_rep=False),
        donate_argnums=donate, keep_unused=True)

    from jax.sharding import NamedSharding
    sh = NamedSharding(mesh, PartitionSpec("core"))
    comp = _Compiled(sharded, in_names, out_avals,
                     jax.device_put(srcg, sh), jax.device_put(dstl, sh))
    _CACHE[key] = comp
    return comp


def kernel(x_inp, node_type, edge_index, edge_type, W_self, W_word,
           att_src_word, att_dst_word, bias_word, W_cross,
           att_src_lang, att_dst_lang, bias_lang):
    global LAST_RESULTS
    x_inp = np.asarray(x_inp, dtype=np.float32)
    comp = _get_compiled(
        np.asarray(edge_index), np.asarray(edge_type),
        [np.asarray(W_self), np.asarray(W_word), np.asarray(att_src_word),
         np.asarray(att_dst_word), np.asarray(bias_word), np.asarray(W_cross),
         np.asarray(att_src_lang), np.asarray(att_dst_lang),
         np.asarray(bias_lang)])

    # per-row int8 quantization of x (LayerNorm is scale-invariant per row;
    # C-cast truncation is within the error budget and ~2x faster than rint)
    amax = np.abs(x_inp).max(axis=1, keepdims=True)
    np.multiply(x_inp, 126.99 / np.maximum(amax, 1e-30), out=comp.xtmp)
    comp.q_buf[:N] = comp.xtmp

    by_name = {"x_sh": comp.q_buf, "srcg": comp.srcg_dev,
               "dstl": comp.dstl_dev}
    args = [by_name[n] for n in comp.in_names]
    if comp.next_seed is None:
        seeds = [np.zeros((M * a.shape[0],) + tuple(a.shape[1:]), a.dtype)
                 for a in comp.out_avals]
    else:
        seeds = comp.next_seed
    out_arrs = comp.sharded(*args, *seeds)
    comp.next_seed = list(out_arrs)
    q = np.asarray(out_arrs[0])          # [M*S, D] int8 delta
    sc = np.asarray(out_arrs[1])         # [M*S, 1] f32 row absmax
    LAST_RESULTS = None
    delta = q[:N].astype(np.float32)
    delta *= sc[:N] / 126.99
    delta += x_inp
    return delta
